# revision 43
# baseline (speedup 1.0000x reference)
"""Multi-head attention (B=2, S=2048, D=1024, H=16) on 8 TRN2 NeuronCores.

Sharding: data parallel on batch (2) x tensor parallel on heads (4 groups of
4 heads).  Core c handles batch c//4, heads 4*(c%4) .. 4*(c%4)+4.  Each core
computes q/k/v projections for its 256 output dims, attention for its 4
heads, and a partial (row-parallel) output projection.  The host sums the 4
partials per batch (plus the out2 tail pieces) and adds b_o.

Per-core kernel (projections/scores bf16, PV fp8e4m3 DoubleRow, fp32 PSUM):
  - qT/kT d-major [256, 2048]; v s-major in fp8 with a ones column at dd=64
    per head (the PV matmul then also emits softmax denominators), packed as
    two j-planes per DoubleRow stationary so one PV matmul covers K=256.
  - scores are computed transposed (S[j, i] = k_j . q_i): no transposes
    anywhere.  The two heads of a pair run as K=64 matmuls on distinct PE
    row-groups (base partitions 0/64) writing the two halves of one shared
    PSUM tile - their drains overlap, so a pair costs ~1.25x one matmul.
  - softmax exp runs on the Scalar engine straight out of PSUM, writing fp8
    E tiles; for DVE_STEPS of the 16 j-steps, exp is instead approximated on
    the Vector engine via the Schraudolph trick (round(S*8/ln2 + 55.55) as
    int8 IS the fp8 bit pattern of ~exp(S)), shedding scalar-engine load.
  - the jt loop is software-pipelined (scores for jt+1 are emitted before
    the PV of jt) so the in-order PE queue never waits on a just-issued exp.
  - i-chunk 512: S-pair tile [128,1024] double-buffered (4 banks) + three
    [128,512] O accumulators (3) + one filler bank = exactly 8 PSUM banks.
  - projections / output-projection groups are emitted as fillers inside the
    attention steps so the PE works while the Scalar engine streams exps.
  - input DMAs are split per k-tile and spread across the sync/gpsimd/scalar
    queues (one queue serializes ~600ns per trigger); the o-projection for
    the tail s-tiles 12-15 is split per kt2-half around the final attn-norm
    chain, with the second halves summed host-side from out2.

Measured (8-core SPMD, fast clock state): ~203us, rel err 1.53e-2
(all-bf16 PV8=False fallback: ~216us, rel err 2.2e-3; gate 2e-2).
"""

import os

import numpy as np
import ml_dtypes

B, S, D = 2, 2048, 1024
H, DH = 16, 64
N_CORES = 8
HPC = 4  # heads per core
DL = HPC * DH  # 256 local dims per core
KT = D // 128  # 8 k-tiles
ST = S // 128  # 16 s-tiles (also j-tiles)
IC = 512  # i-chunk (query chunk)
NIC = S // IC

_BF16 = ml_dtypes.bfloat16

# fp8-DoubleRow PV: E and v in fp8e4m3, PV matmuls cover two j-tiles per
# instruction (K=256 via the two fp8 k-planes).  Halves PV tensor time for
# ~1.5e-2 rel err (vs 2.2e-3 all-bf16; gate is 2e-2).
PV8 = True

_nc_cache = None


def _build_nc():
    from contextlib import ExitStack

    import concourse.mybir as mybir
    import concourse.tile as tile
    from concourse import bacc

    f32 = mybir.dt.float32
    bf16 = mybir.dt.bfloat16
    f8 = mybir.dt.float8e4
    i8 = mybir.dt.int8
    Alu = mybir.AluOpType
    Act = mybir.ActivationFunctionType
    DRMODE = mybir.MatmulPerfMode.DoubleRow

    nc = bacc.Bacc("TRN2", target_bir_lowering=False, debug=False, enable_asserts=False)

    xT_d = nc.dram_tensor("xT", (D, S), bf16, kind="ExternalInput")  # [k, s]
    wq_d = nc.dram_tensor("wq", (D, DL), bf16, kind="ExternalInput")  # [k, dl]
    wk_d = nc.dram_tensor("wk", (D, DL), bf16, kind="ExternalInput")
    wv_d = nc.dram_tensor("wv", (D, DL), bf16, kind="ExternalInput")
    wo_d = nc.dram_tensor("wo", (DL, D), bf16, kind="ExternalInput")  # [dl, o]
    bqk_d = nc.dram_tensor("bqk", (128, 4), f32, kind="ExternalInput")
    bv_d = nc.dram_tensor("bv", (128, DL), f32, kind="ExternalInput")
    out_d = nc.dram_tensor("out", (S, D), f32, kind="ExternalOutput")
    # second half (kt2=1) of the o-projection for the tail s-tiles 12..15 —
    # summed into out rows 1536:2048 host-side, so the tail matmuls can split
    # around the last attn_norm chain instead of serializing after it.
    out2_d = nc.dram_tensor("out2", (4 * 128, D), f32, kind="ExternalOutput")

    with tile.TileContext(nc) as tc, ExitStack() as ctx:
        consts = ctx.enter_context(tc.tile_pool(name="consts", bufs=1))
        xbf = consts.tile([128, KT, S], bf16)  # [p, kt, s]
        wq_sb = consts.tile([128, KT, DL], bf16)
        wk_sb = consts.tile([128, KT, DL], bf16)
        wv_sb = consts.tile([128, KT, DL], bf16)
        wo_sb = consts.tile([128, 2, D], bf16)  # [p, kt2, o]
        bqk_sb = consts.tile([128, 4], f32)
        bv_sb = consts.tile([128, DL], f32)
        qT = consts.tile([128, 2, S], bf16)  # [p, mt, s]
        kT = consts.tile([128, 2, S], bf16)
        # v (s-major) + ones column at 64 (so the PV matmul also emits
        # softmax denominators).  bf16 path: zero-padded to 128 cols per
        # (jt, h).  fp8 path: [p, jp, plane, h, dd] with two j-planes per
        # DoubleRow stationary (dd padded to 68 for alignment).
        if PV8:
            vaug = consts.tile([128, ST // 2, 2, HPC, 68], f8)
        else:
            vaug = consts.tile([128, ST, HPC, 128], bf16)  # [p(j), jt, h, dd]
        aoT = consts.tile([128, 2, S], bf16)  # attn-out transposed [p, kt2, s]

        # Preload the exp activation table set (~2.7us) during the DMA
        # lead-in so the first real softmax exp doesn't pay for it.
        warm = consts.tile([128, 8], f32)
        nc.gpsimd.memset(warm[:], 0.0)
        nc.scalar.activation(warm[:], warm[:], Act.Exp)

        # ---- input DMAs: per-kt interleaved wk/x slices so the first
        # projection matmul (which consumes kt sequentially) starts as soon
        # as slice 0 lands instead of after the full wk + x chunk.  Triggers
        # are spread across engine queues — a sync-queue trigger costs
        # ~600ns, so 40 triggers on one queue would serialize the startup.
        for kt in range(KT):
            nc.sync.dma_start(
                wk_sb[:, kt, :], wk_d.ap()[kt * 128 : (kt + 1) * 128, :]
            )
            nc.gpsimd.dma_start(
                xbf[:, kt, 0:512], xT_d.ap()[kt * 128 : (kt + 1) * 128, 0:512]
            )
        for kt in range(KT):
            eng = nc.sync if kt % 2 == 0 else nc.gpsimd
            eng.dma_start(
                xbf[:, kt, 512:1024], xT_d.ap()[kt * 128 : (kt + 1) * 128, 512:1024]
            )
        nc.scalar.dma_start(wq_sb[:], wq_d.ap().rearrange("(kt p) m -> p kt m", p=128))
        nc.scalar.dma_start(bqk_sb[:], bqk_d.ap())
        nc.scalar.dma_start(wv_sb[:], wv_d.ap().rearrange("(kt p) m -> p kt m", p=128))
        nc.scalar.dma_start(bv_sb[:], bv_d.ap())

        if PV8:
            nc.gpsimd.memset(vaug[:, :, :, :, DH:], 0.0)
            nc.gpsimd.memset(vaug[:, :, :, :, DH : DH + 1], 1.0)
        else:
            nc.gpsimd.memset(vaug[:, :, :, DH + 1 :], 0.0)
            nc.gpsimd.memset(vaug[:, :, :, DH : DH + 1], 1.0)

        for sc in range(2, 4):
            for kt in range(KT):
                eng = nc.sync if (sc * KT + kt) % 2 == 0 else nc.gpsimd
                eng.dma_start(
                    xbf[:, kt, sc * 512 : (sc + 1) * 512],
                    xT_d.ap()[kt * 128 : (kt + 1) * 128, sc * 512 : (sc + 1) * 512],
                )
        nc.sync.dma_start(wo_sb[:], wo_d.ap().rearrange("(kt p) m -> p kt m", p=128))

        # Output stores alternate between the sync and gpsimd DMA queues —
        # a single queue serializes ~10MB of result transfers (~28us) and
        # its backlog was draining for ~10us after the last matmul.
        dctr = [0]

        def out_dma(dst, src):
            dctr[0] += 1
            eng = nc.sync if dctr[0] % 2 else nc.gpsimd
            eng.dma_start(dst, src)

        ps = ctx.enter_context(tc.tile_pool(name="ps", bufs=2, space="PSUM"))
        op_ = ctx.enter_context(tc.tile_pool(name="op", bufs=3, space="PSUM"))
        fp = ctx.enter_context(tc.tile_pool(name="fp", bufs=1, space="PSUM"))
        ep = ctx.enter_context(tc.tile_pool(name="ep", bufs=8))
        rp = ctx.enter_context(tc.tile_pool(name="rp", bufs=3))
        tp = ctx.enter_context(tc.tile_pool(name="tp", bufs=3))
        osb = ctx.enter_context(tc.tile_pool(name="osb", bufs=3))

        def qk_proj(proj, mt, c, alt=False):
            """q (proj=0) / k (proj=1) projection, one 512-col chunk."""
            w_sb = wq_sb if proj == 0 else wk_sb
            dst_all = qT if proj == 0 else kT
            pool, tg = (op_, "O") if alt else (fp, "f")
            p = pool.tile([128, 512], f32, tag=tg)
            for kt in range(KT):
                nc.tensor.matmul(
                    p[:],
                    w_sb[:, kt, mt * 128 : (mt + 1) * 128],
                    xbf[:, kt, c * 512 : (c + 1) * 512],
                    start=(kt == 0),
                    stop=(kt == KT - 1),
                )
            dst = dst_all[:, mt, c * 512 : (c + 1) * 512]
            bias_ap = bqk_sb[:, proj * 2 + mt : proj * 2 + mt + 1]
            if proj == 0:
                nc.vector.tensor_scalar(dst, p[:], bias_ap, 0.125, Alu.add, Alu.mult)
            else:
                nc.vector.tensor_scalar(dst, p[:], bias_ap, None, Alu.add)

        def v_proj(st):
            pool, tg = (fp, "f") if st % 2 == 0 else (op_, "O")
            p = pool.tile([128, 512], f32, tag=tg)
            for kt in range(KT):
                nc.tensor.matmul(
                    p[:, 0:DL],
                    xbf[:, kt, st * 128 : (st + 1) * 128],
                    wv_sb[:, kt, :],
                    start=(kt == 0),
                    stop=(kt == KT - 1),
                )
            if PV8:
                dst = vaug[:, st // 2, st % 2, :, 0:DH]
            else:
                dst = vaug[:, st, :, 0:DH]
            nc.vector.tensor_tensor(
                dst,
                p[:, 0:DL].rearrange("p (h d) -> p h d", h=HPC),
                bv_sb[:].rearrange("p (h d) -> p h d", h=HPC),
                Alu.add,
            )

        def o_proj_half(st, oc, kt2):
            """One kt2 half of the o-projection for a tail s-tile: K=128
            matmul -> evict -> DMA.  kt2=0 (pair-0 heads) goes to out rows
            (as fillers, aoT[:,0] for ic3 is ready one pair_ic early);
            kt2=1 goes to out2 and is summed host-side."""
            pool, tg = (fp, "f") if (st + oc) % 2 == 0 else (op_, "O")
            pso = pool.tile([128, 512], f32, tag=tg)
            nc.tensor.matmul(
                pso[:],
                aoT[:, kt2, st * 128 : (st + 1) * 128],
                wo_sb[:, kt2, oc * 512 : (oc + 1) * 512],
                start=True,
                stop=True,
            )
            stg = osb.tile([128, 512], f32, tag="oh")
            if kt2 == 0:
                nc.vector.tensor_copy(stg[:], pso[:])
                nc.sync.dma_start(
                    out_d.ap()[st * 128 : (st + 1) * 128,
                               oc * 512 : (oc + 1) * 512], stg[:])
            else:
                if (st + oc) % 2 == 0:
                    nc.scalar.copy(stg[:], pso[:])
                else:
                    nc.vector.tensor_copy(stg[:], pso[:])
                row = (st - 12) * 128
                out_dma(out2_d.ap()[row : row + 128,
                                    oc * 512 : (oc + 1) * 512], stg[:])

        def o_proj_chunk(st, oc):
            pso = fp.tile([128, 512], f32, tag="f")
            for kt2 in range(2):
                nc.tensor.matmul(
                    pso[:],
                    aoT[:, kt2, st * 128 : (st + 1) * 128],
                    wo_sb[:, kt2, oc * 512 : (oc + 1) * 512],
                    start=(kt2 == 0),
                    stop=(kt2 == 1),
                )
            stg = osb.tile([128, 512], f32, tag="oh")
            nc.vector.tensor_copy(stg[:], pso[:])
            nc.sync.dma_start(
                out_d.ap()[st * 128 : (st + 1) * 128, oc * 512 : (oc + 1) * 512],
                stg[:],
            )

        ones64 = consts.tile([1, DH], f32)
        nc.gpsimd.memset(ones64[:], 1.0)
        f32r = mybir.dt.float32r

        def attn_norm(h, ic, O, pe_bcast=False):
            # pe_bcast flags the LAST pair's norms: those sit on the tail
            # critical path, so the chain is split into i-halves (the den
            # copy on the now-idle Scalar engine) to pipeline its latency
            # and let the tail o-projection start on the first half.
            pb, mt = 64 * (h % 2), h // 2
            HW_ = IC // 2 if pe_bcast else IC
            for hf in range(IC // HW_):
                lo, hi = hf * HW_, (hf + 1) * HW_
                den = rp.tile([1, IC], f32, tag="den")
                if pe_bcast:
                    nc.scalar.copy(den[:, 0:HW_], O[DH : DH + 1, lo:hi])
                else:
                    nc.vector.tensor_copy(den[:, 0:HW_], O[DH : DH + 1, lo:hi])
                recip = rp.tile([1, IC], f32, tag="r")
                nc.vector.reciprocal_approx_fast(recip[:, 0:HW_], den[:, 0:HW_])
                rb = rp.tile([64, IC], f32, tag="rb")
                nc.gpsimd.partition_broadcast(rb[:, 0:HW_], recip[:, 0:HW_])
                base = ic * IC + lo
                if pb == 0:
                    # even heads land on the same partitions as O rows 0..63
                    # — write straight into aoT, no SBUF->SBUF DMA needed.
                    nc.vector.tensor_tensor(
                        aoT[0:64, mt, base : base + HW_], O[0:DH, lo:hi],
                        rb[:, 0:HW_], Alu.mult,
                    )
                else:
                    tmp = tp.tile([64, IC], bf16, tag="t")
                    nc.vector.tensor_tensor(
                        tmp[:, 0:HW_], O[0:DH, lo:hi], rb[:, 0:HW_], Alu.mult
                    )
                    nc.sync.dma_start(
                        aoT[pb : pb + 64, mt, base : base + HW_], tmp[:, 0:HW_]
                    )

        # Schraudolph fast-exp constants: round(S*SCA + SCB) as int16 IS the
        # bf16 bit pattern of ~exp(S) (max rel err 4.1%, std 1.8%, ~zero
        # mean).  Emitted on the Vector engine for DVE_STEPS of the 16 jt
        # steps so the Scalar engine (the per-step pacer) sheds 1/4 of the
        # exp stream.
        SCA = float(128.0 / np.log(2.0))
        SCB = float(127 * 128.0 - 7.3)
        SCA8 = float(8.0 / np.log(2.0))
        SCB8 = float(56.0 - 0.45)
        DVE_STEPS = (2, 6, 9, 13) if PV8 else ()

        def pair_ic(pair, ic, fillers, last=False):
            """Attention for head pair (2*pair, 2*pair+1) on query chunk ic.
            fillers: {jt: [callable, ...]} emitted just before that step.
            Software-pipelined: scores for jt+1 are emitted before the PV of
            jt, so the in-order PE queue never sits behind a PV that waits
            on a just-issued exp."""
            hA, hB = 2 * pair, 2 * pair + 1
            OA = op_.tile([128, IC], f32, tag="O")
            OB = op_.tile([128, IC], f32, tag="O")

            def s_step(jt, E, pl):
                Sp = ps.tile([128, 2 * IC], f32, tag="S")
                nc.tensor.matmul(
                    Sp[:, 0:IC],
                    kT[0:64, pair, jt * 128 : (jt + 1) * 128],
                    qT[0:64, pair, ic * IC : (ic + 1) * IC],
                    start=True,
                    stop=True,
                )
                nc.tensor.matmul(
                    Sp[:, IC : 2 * IC],
                    kT[64:128, pair, jt * 128 : (jt + 1) * 128],
                    qT[64:128, pair, ic * IC : (ic + 1) * IC],
                    start=True,
                    stop=True,
                )
                dst = E[:, pl, :] if PV8 else E[:]
                if jt in DVE_STEPS:
                    nc.vector.tensor_scalar(
                        dst.bitcast(i8 if PV8 else mybir.dt.int16), Sp[:],
                        SCA8 if PV8 else SCA, SCB8 if PV8 else SCB,
                        Alu.mult, Alu.add,
                    )
                else:
                    nc.scalar.activation(dst, Sp[:], Act.Exp)

            def new_E():
                if PV8:
                    E = ep.tile([128, 2, 2 * IC], f8, tag="E")
                else:
                    E = ep.tile([128, 2 * IC], bf16, tag="E")
                return E

            def pv_step(jt, E):
                # bf16: one j-tile per call.  fp8 DoubleRow: jt is a j-PAIR
                # index, E carries both j-planes, K=256 per matmul.
                if PV8:
                    lastp = ST // 2 - 1
                    nc.tensor.matmul(
                        OA[0:DH + 1, :],
                        vaug[:, jt, :, hA, 0 : DH + 1],
                        E[:, :, 0:IC],
                        start=(jt == 0),
                        stop=(jt == lastp),
                        perf_mode=DRMODE,
                    )
                    nc.tensor.matmul(
                        OB[0:DH + 1, :],
                        vaug[:, jt, :, hB, 0 : DH + 1],
                        E[:, :, IC : 2 * IC],
                        start=(jt == 0),
                        stop=(jt == lastp),
                        perf_mode=DRMODE,
                    )
                else:
                    nc.tensor.matmul(
                        OA[:],
                        vaug[:, jt, hA, :],
                        E[:, 0:IC],
                        start=(jt == 0),
                        stop=(jt == ST - 1),
                    )
                    nc.tensor.matmul(
                        OB[:],
                        vaug[:, jt, hB, :],
                        E[:, IC : 2 * IC],
                        start=(jt == 0),
                        stop=(jt == ST - 1),
                    )

            if PV8:
                # jt-pair pipeline: S(2jp)/S(2jp+1) fill the two planes of
                # E-tile jp; the DR PV for pair jp-1 is emitted two s_steps
                # later so exp always has slack.
                Etiles = {}
                for jt in range(ST):
                    for f in fillers.get(jt, ()):
                        f()
                    jp = jt // 2
                    if jt % 2 == 0:
                        Etiles[jp] = new_E()
                    s_step(jt, Etiles[jp], jt % 2)
                    if jt % 2 == 1 and jp >= 1:
                        pv_step(jp - 1, Etiles.pop(jp - 1))
                pv_step(ST // 2 - 1, Etiles.pop(ST // 2 - 1))
            else:
                for f in fillers.get(0, ()):
                    f()
                Eprev = new_E()
                s_step(0, Eprev, 0)
                for jt in range(1, ST):
                    for f in fillers.get(jt, ()):
                        f()
                    Ecur = new_E()
                    s_step(jt, Ecur, 0)
                    pv_step(jt - 1, Eprev)
                    Eprev = Ecur
                pv_step(ST - 1, Eprev)
            attn_norm(hA, ic, OA, pe_bcast=last)
            attn_norm(hB, ic, OB, pe_bcast=last)

        # ---- emission schedule ----
        qk_proj(1, 0, 0)
        qk_proj(0, 0, 0, alt=True)
        v_proj(0)
        v_proj(1)
        F = lambda *fs: list(fs)
        p0i0 = {jt: F(lambda st=jt + 2: v_proj(st)) for jt in range(ST - 2)}
        for jt, c in ((2, 1), (5, 2), (9, 3)):
            p0i0[jt] = [lambda c=c: qk_proj(1, 0, c)] + p0i0[jt]
        p0i0[12] = [lambda: qk_proj(0, 0, 1)] + p0i0[12]
        pair_ic(0, 0, p0i0)
        pair_ic(0, 1, {
            4: F(lambda: qk_proj(1, 1, 0)),
            6: F(lambda: qk_proj(1, 1, 1)),
            8: F(lambda: qk_proj(1, 1, 2)),
            10: F(lambda: qk_proj(1, 1, 3)),
            13: F(lambda: qk_proj(0, 1, 0)),
        })
        pair_ic(1, 0, {
            4: F(lambda: qk_proj(0, 1, 1)),
            9: F(lambda: qk_proj(0, 0, 2)),
        })
        pair_ic(1, 1, {
            4: F(lambda: qk_proj(0, 1, 2)),
            9: F(lambda: qk_proj(0, 0, 3)),
            12: F(lambda: qk_proj(0, 1, 3)),
        })
        pair_ic(0, 2, {5 + i: F(lambda st=(i + 2) // 2, oc=i % 2: o_proj_chunk(st, oc))
                       for i in range(6)})
        pair_ic(1, 2, {4 + i: F(lambda st=4 + i // 2, oc=i % 2: o_proj_chunk(st, oc))
                       for i in range(8)})
        pair_ic(0, 3, {4 + i: F(lambda st=8 + i // 2, oc=i % 2: o_proj_chunk(st, oc))
                       for i in range(8)})
        p13 = {4: F(lambda: o_proj_chunk(0, 0)),
               8: F(lambda: o_proj_chunk(0, 1))}
        slots = (5, 6, 7, 9, 10, 11, 12, 13)
        for i in range(8):
            st, oc = 12 + i // 2, i % 2
            p13.setdefault(slots[i], []).append(
                lambda st=st, oc=oc: o_proj_half(st, oc, 0))
        pair_ic(1, 3, p13, last=True)
        # Keep the PE active through the final norm-chain gap with dummy
        # matmuls (never read) — otherwise the HAM clock manager drops to
        # 4/8 duty and the tail o-projection runs ~1.6x slower.
        for r in range(10):
            pool, tg = (fp, "f") if r % 2 == 0 else (op_, "O")
            dmy = pool.tile([128, 512], f32, tag=tg)
            nc.tensor.matmul(
                dmy[:], wo_sb[:, 0, 0:128], wo_sb[:, 1, 0:512],
                start=True, stop=True,
            )
        for st in range(12, ST):
            o_proj_half(st, 0, 1)
            o_proj_half(st, 1, 1)

    nc.compile()
    return nc


def _get_nc():
    global _nc_cache
    if _nc_cache is None:
        _nc_cache = _build_nc()
    return _nc_cache


def _prepare_in_maps(x, W_q, b_q, W_k, b_k, W_v, b_v, W_o, b_o):
    in_maps = []
    for c in range(N_CORES):
        b, g = c // 4, c % 4
        rows = slice(DL * g, DL * g + DL)
        bqk = np.stack(
            [
                b_q[DL * g : DL * g + 128],
                b_q[DL * g + 128 : DL * g + 256],
                b_k[DL * g : DL * g + 128],
                b_k[DL * g + 128 : DL * g + 256],
            ],
            axis=1,
        ).astype(np.float32)
        in_maps.append(
            {
                "xT": np.ascontiguousarray(x[b].T).astype(_BF16),
                "wq": np.ascontiguousarray(W_q[rows].T).astype(_BF16),
                "wk": np.ascontiguousarray(W_k[rows].T).astype(_BF16),
                "wv": np.ascontiguousarray(W_v[rows].T).astype(_BF16),
                "wo": np.ascontiguousarray(W_o[:, rows].T).astype(_BF16),
                "bqk": np.ascontiguousarray(bqk),
                "bv": np.ascontiguousarray(
                    np.broadcast_to(b_v[rows], (128, DL))
                ).astype(np.float32),
            }
        )
    return in_maps


def _assemble(results, b_o):
    out = np.empty((B, S, D), dtype=np.float32)
    for b in range(B):
        acc = results[4 * b]["out"].astype(np.float32).copy()
        acc[12 * 128 :] += results[4 * b]["out2"]
        for g in range(1, 4):
            acc += results[4 * b + g]["out"]
            acc[12 * 128 :] += results[4 * b + g]["out2"]
        out[b] = acc + b_o[None, :].astype(np.float32)
    return out


def kernel(x, W_q, b_q, W_k, b_k, W_v, b_v, W_o, b_o):
    from concourse.bass_utils import run_bass_kernel_spmd

    x = np.asarray(x, dtype=np.float32)
    nc = _get_nc()
    in_maps = _prepare_in_maps(
        x,
        np.asarray(W_q, np.float32),
        np.asarray(b_q, np.float32),
        np.asarray(W_k, np.float32),
        np.asarray(b_k, np.float32),
        np.asarray(W_v, np.float32),
        np.asarray(b_v, np.float32),
        np.asarray(W_o, np.float32),
        np.asarray(b_o, np.float32),
    )
    res = run_bass_kernel_spmd(nc, in_maps, core_ids=list(range(N_CORES)))
    return _assemble(res.results, np.asarray(b_o, np.float32))



# revision 44
# speedup vs baseline: 1.0040x; 1.0040x over previous
"""Multi-head attention (B=2, S=2048, D=1024, H=16) on 8 TRN2 NeuronCores.

Sharding: data parallel on batch (2) x tensor parallel on heads (4 groups of
4 heads).  Core c handles batch c//4, heads 4*(c%4) .. 4*(c%4)+4.  Each core
computes q/k/v projections for its 256 output dims, attention for its 4
heads, and a partial (row-parallel) output projection.  The host sums the 4
partials per batch (plus the out2 tail pieces) and adds b_o.

Per-core kernel (projections/scores bf16, PV fp8e4m3 DoubleRow, fp32 PSUM):
  - qT/kT d-major [256, 2048]; v s-major in fp8 with a ones column at dd=64
    per head (the PV matmul then also emits softmax denominators), packed as
    two j-planes per DoubleRow stationary so one PV matmul covers K=256.
  - scores are computed transposed (S[j, i] = k_j . q_i): no transposes
    anywhere.  The two heads of a pair run as K=64 matmuls on distinct PE
    row-groups (base partitions 0/64) writing the two halves of one shared
    PSUM tile - their drains overlap, so a pair costs ~1.25x one matmul.
  - softmax exp runs on the Scalar engine straight out of PSUM, writing fp8
    E tiles; for DVE_STEPS of the 16 j-steps, exp is instead approximated on
    the Vector engine via the Schraudolph trick (round(S*8/ln2 + 55.55) as
    int8 IS the fp8 bit pattern of ~exp(S)), shedding scalar-engine load.
  - the jt loop is software-pipelined (scores for jt+1 are emitted before
    the PV of jt) so the in-order PE queue never waits on a just-issued exp.
  - i-chunk 512: S-pair tile [128,1024] double-buffered (4 banks) + three
    [128,512] O accumulators (3) + one filler bank = exactly 8 PSUM banks.
  - projections / output-projection groups are emitted as fillers inside the
    attention steps so the PE works while the Scalar engine streams exps.
  - input DMAs are split per k-tile and spread across the sync/gpsimd/scalar
    queues (one queue serializes ~600ns per trigger); the o-projection for
    the tail s-tiles 12-15 is split per kt2-half around the final attn-norm
    chain, with the second halves summed host-side from out2.

Measured (8-core SPMD, fast clock state): ~203us, rel err 1.53e-2
(all-bf16 PV8=False fallback: ~216us, rel err 2.2e-3; gate 2e-2).
"""

import os

import numpy as np
import ml_dtypes

B, S, D = 2, 2048, 1024
H, DH = 16, 64
N_CORES = 8
HPC = 4  # heads per core
DL = HPC * DH  # 256 local dims per core
KT = D // 128  # 8 k-tiles
ST = S // 128  # 16 s-tiles (also j-tiles)
IC = 512  # i-chunk (query chunk)
NIC = S // IC

_BF16 = ml_dtypes.bfloat16

# fp8-DoubleRow PV: E and v in fp8e4m3, PV matmuls cover two j-tiles per
# instruction (K=256 via the two fp8 k-planes).  Halves PV tensor time for
# ~1.5e-2 rel err (vs 2.2e-3 all-bf16; gate is 2e-2).
PV8 = True

_nc_cache = None


def _build_nc():
    from contextlib import ExitStack

    import concourse.mybir as mybir
    import concourse.tile as tile
    from concourse import bacc

    f32 = mybir.dt.float32
    bf16 = mybir.dt.bfloat16
    f8 = mybir.dt.float8e4
    i8 = mybir.dt.int8
    Alu = mybir.AluOpType
    Act = mybir.ActivationFunctionType
    DRMODE = mybir.MatmulPerfMode.DoubleRow

    nc = bacc.Bacc("TRN2", target_bir_lowering=False, debug=False, enable_asserts=False)

    xT_d = nc.dram_tensor("xT", (D, S), bf16, kind="ExternalInput")  # [k, s]
    wq_d = nc.dram_tensor("wq", (D, DL), bf16, kind="ExternalInput")  # [k, dl]
    wk_d = nc.dram_tensor("wk", (D, DL), bf16, kind="ExternalInput")
    wv_d = nc.dram_tensor("wv", (D, DL), bf16, kind="ExternalInput")
    wo_d = nc.dram_tensor("wo", (DL, D), bf16, kind="ExternalInput")  # [dl, o]
    bqk_d = nc.dram_tensor("bqk", (128, 4), f32, kind="ExternalInput")
    bv_d = nc.dram_tensor("bv", (128, DL), f32, kind="ExternalInput")
    out_d = nc.dram_tensor("out", (S, D), f32, kind="ExternalOutput")
    # second half (kt2=1) of the o-projection for the tail s-tiles 12..15 —
    # summed into out rows 1536:2048 host-side, so the tail matmuls can split
    # around the last attn_norm chain instead of serializing after it.
    out2_d = nc.dram_tensor("out2", (4 * 128, D), f32, kind="ExternalOutput")

    with tile.TileContext(nc) as tc, ExitStack() as ctx:
        consts = ctx.enter_context(tc.tile_pool(name="consts", bufs=1))
        xbf = consts.tile([128, KT, S], bf16)  # [p, kt, s]
        wq_sb = consts.tile([128, KT, DL], bf16)
        wk_sb = consts.tile([128, KT, DL], bf16)
        wv_sb = consts.tile([128, KT, DL], bf16)
        wo_sb = consts.tile([128, 2, D], bf16)  # [p, kt2, o]
        bqk_sb = consts.tile([128, 4], f32)
        bv_sb = consts.tile([128, DL], f32)
        qT = consts.tile([128, 2, S], bf16)  # [p, mt, s]
        kT = consts.tile([128, 2, S], bf16)
        # v (s-major) + ones column at 64 (so the PV matmul also emits
        # softmax denominators).  bf16 path: zero-padded to 128 cols per
        # (jt, h).  fp8 path: [p, jp, plane, h, dd] with two j-planes per
        # DoubleRow stationary (dd padded to 68 for alignment).
        if PV8:
            vaug = consts.tile([128, ST // 2, 2, HPC, 68], f8)
        else:
            vaug = consts.tile([128, ST, HPC, 128], bf16)  # [p(j), jt, h, dd]
        aoT = consts.tile([128, 2, S], bf16)  # attn-out transposed [p, kt2, s]

        # Preload the exp activation table set (~2.7us) during the DMA
        # lead-in so the first real softmax exp doesn't pay for it.
        warm = consts.tile([128, 8], f32)
        nc.gpsimd.memset(warm[:], 0.0)
        nc.scalar.activation(warm[:], warm[:], Act.Exp)

        # ---- input DMAs: per-kt interleaved wk/x slices so the first
        # projection matmul (which consumes kt sequentially) starts as soon
        # as slice 0 lands instead of after the full wk + x chunk.  Triggers
        # are spread across engine queues — a sync-queue trigger costs
        # ~600ns, so 40 triggers on one queue would serialize the startup.
        for kt in range(KT):
            nc.sync.dma_start(
                wk_sb[:, kt, :], wk_d.ap()[kt * 128 : (kt + 1) * 128, :]
            )
            nc.gpsimd.dma_start(
                xbf[:, kt, 0:512], xT_d.ap()[kt * 128 : (kt + 1) * 128, 0:512]
            )
        for kt in range(KT):
            eng = nc.sync if kt % 2 == 0 else nc.gpsimd
            eng.dma_start(
                xbf[:, kt, 512:1024], xT_d.ap()[kt * 128 : (kt + 1) * 128, 512:1024]
            )
        nc.scalar.dma_start(wq_sb[:], wq_d.ap().rearrange("(kt p) m -> p kt m", p=128))
        nc.scalar.dma_start(bqk_sb[:], bqk_d.ap())
        nc.scalar.dma_start(wv_sb[:], wv_d.ap().rearrange("(kt p) m -> p kt m", p=128))
        nc.scalar.dma_start(bv_sb[:], bv_d.ap())

        if PV8:
            nc.gpsimd.memset(vaug[:, :, :, :, DH:], 0.0)
            nc.gpsimd.memset(vaug[:, :, :, :, DH : DH + 1], 1.0)
        else:
            nc.gpsimd.memset(vaug[:, :, :, DH + 1 :], 0.0)
            nc.gpsimd.memset(vaug[:, :, :, DH : DH + 1], 1.0)

        for sc in range(2, 4):
            for kt in range(KT):
                eng = nc.sync if (sc * KT + kt) % 2 == 0 else nc.gpsimd
                eng.dma_start(
                    xbf[:, kt, sc * 512 : (sc + 1) * 512],
                    xT_d.ap()[kt * 128 : (kt + 1) * 128, sc * 512 : (sc + 1) * 512],
                )
        nc.sync.dma_start(wo_sb[:], wo_d.ap().rearrange("(kt p) m -> p kt m", p=128))

        # Output stores alternate between the sync and gpsimd DMA queues —
        # a single queue serializes ~10MB of result transfers (~28us) and
        # its backlog was draining for ~10us after the last matmul.
        dctr = [0]

        def out_dma(dst, src):
            dctr[0] += 1
            eng = nc.sync if dctr[0] % 2 else nc.gpsimd
            eng.dma_start(dst, src)

        ps = ctx.enter_context(tc.tile_pool(name="ps", bufs=2, space="PSUM"))
        op_ = ctx.enter_context(tc.tile_pool(name="op", bufs=3, space="PSUM"))
        fp = ctx.enter_context(tc.tile_pool(name="fp", bufs=1, space="PSUM"))
        ep = ctx.enter_context(tc.tile_pool(name="ep", bufs=8))
        rp = ctx.enter_context(tc.tile_pool(name="rp", bufs=3))
        tp = ctx.enter_context(tc.tile_pool(name="tp", bufs=3))
        osb = ctx.enter_context(tc.tile_pool(name="osb", bufs=3))

        def qk_proj(proj, mt, c, alt=False):
            """q (proj=0) / k (proj=1) projection, one 512-col chunk."""
            w_sb = wq_sb if proj == 0 else wk_sb
            dst_all = qT if proj == 0 else kT
            pool, tg = (op_, "O") if alt else (fp, "f")
            p = pool.tile([128, 512], f32, tag=tg)
            for kt in range(KT):
                nc.tensor.matmul(
                    p[:],
                    w_sb[:, kt, mt * 128 : (mt + 1) * 128],
                    xbf[:, kt, c * 512 : (c + 1) * 512],
                    start=(kt == 0),
                    stop=(kt == KT - 1),
                )
            dst = dst_all[:, mt, c * 512 : (c + 1) * 512]
            bias_ap = bqk_sb[:, proj * 2 + mt : proj * 2 + mt + 1]
            if proj == 0:
                nc.vector.tensor_scalar(dst, p[:], bias_ap, 0.125, Alu.add, Alu.mult)
            else:
                nc.vector.tensor_scalar(dst, p[:], bias_ap, None, Alu.add)

        def v_proj(st):
            pool, tg = (fp, "f") if st % 2 == 0 else (op_, "O")
            p = pool.tile([128, 512], f32, tag=tg)
            for kt in range(KT):
                nc.tensor.matmul(
                    p[:, 0:DL],
                    xbf[:, kt, st * 128 : (st + 1) * 128],
                    wv_sb[:, kt, :],
                    start=(kt == 0),
                    stop=(kt == KT - 1),
                )
            if PV8:
                dst = vaug[:, st // 2, st % 2, :, 0:DH]
            else:
                dst = vaug[:, st, :, 0:DH]
            nc.vector.tensor_tensor(
                dst,
                p[:, 0:DL].rearrange("p (h d) -> p h d", h=HPC),
                bv_sb[:].rearrange("p (h d) -> p h d", h=HPC),
                Alu.add,
            )

        def o_proj_half(st, oc, kt2):
            """One kt2 half of the o-projection for a tail s-tile: K=128
            matmul -> evict -> DMA.  kt2=0 (pair-0 heads) goes to out rows
            (as fillers, aoT[:,0] for ic3 is ready one pair_ic early);
            kt2=1 goes to out2 and is summed host-side."""
            pool, tg = (fp, "f") if (st + oc) % 2 == 0 else (op_, "O")
            pso = pool.tile([128, 512], f32, tag=tg)
            nc.tensor.matmul(
                pso[:],
                aoT[:, kt2, st * 128 : (st + 1) * 128],
                wo_sb[:, kt2, oc * 512 : (oc + 1) * 512],
                start=True,
                stop=True,
            )
            stg = osb.tile([128, 512], f32, tag="oh")
            if kt2 == 0:
                nc.vector.tensor_copy(stg[:], pso[:])
                nc.sync.dma_start(
                    out_d.ap()[st * 128 : (st + 1) * 128,
                               oc * 512 : (oc + 1) * 512], stg[:])
            else:
                if (st + oc) % 2 == 0:
                    nc.scalar.copy(stg[:], pso[:])
                else:
                    nc.vector.tensor_copy(stg[:], pso[:])
                row = (st - 12) * 128
                out_dma(out2_d.ap()[row : row + 128,
                                    oc * 512 : (oc + 1) * 512], stg[:])

        def o_proj_chunk(st, oc):
            pso = fp.tile([128, 512], f32, tag="f")
            for kt2 in range(2):
                nc.tensor.matmul(
                    pso[:],
                    aoT[:, kt2, st * 128 : (st + 1) * 128],
                    wo_sb[:, kt2, oc * 512 : (oc + 1) * 512],
                    start=(kt2 == 0),
                    stop=(kt2 == 1),
                )
            stg = osb.tile([128, 512], f32, tag="oh")
            nc.vector.tensor_copy(stg[:], pso[:])
            nc.sync.dma_start(
                out_d.ap()[st * 128 : (st + 1) * 128, oc * 512 : (oc + 1) * 512],
                stg[:],
            )

        ones64 = consts.tile([1, DH], f32)
        nc.gpsimd.memset(ones64[:], 1.0)
        f32r = mybir.dt.float32r

        def attn_norm(h, ic, O, pe_bcast=False):
            # pe_bcast flags the LAST pair's norms: those sit on the tail
            # critical path, so the chain is split into i-halves (the den
            # copy on the now-idle Scalar engine) to pipeline its latency
            # and let the tail o-projection start on the first half.
            pb, mt = 64 * (h % 2), h // 2
            HW_ = IC // 2 if pe_bcast else IC
            for hf in range(IC // HW_):
                lo, hi = hf * HW_, (hf + 1) * HW_
                den = rp.tile([1, IC], f32, tag="den")
                if pe_bcast:
                    nc.scalar.copy(den[:, 0:HW_], O[DH : DH + 1, lo:hi])
                else:
                    nc.vector.tensor_copy(den[:, 0:HW_], O[DH : DH + 1, lo:hi])
                recip = rp.tile([1, IC], f32, tag="r")
                nc.vector.reciprocal_approx_fast(recip[:, 0:HW_], den[:, 0:HW_])
                rb = rp.tile([64, IC], f32, tag="rb")
                nc.gpsimd.partition_broadcast(rb[:, 0:HW_], recip[:, 0:HW_])
                base = ic * IC + lo
                if pb == 0:
                    # even heads land on the same partitions as O rows 0..63
                    # — write straight into aoT, no SBUF->SBUF DMA needed.
                    nc.vector.tensor_tensor(
                        aoT[0:64, mt, base : base + HW_], O[0:DH, lo:hi],
                        rb[:, 0:HW_], Alu.mult,
                    )
                else:
                    tmp = tp.tile([64, IC], bf16, tag="t")
                    nc.vector.tensor_tensor(
                        tmp[:, 0:HW_], O[0:DH, lo:hi], rb[:, 0:HW_], Alu.mult
                    )
                    nc.sync.dma_start(
                        aoT[pb : pb + 64, mt, base : base + HW_], tmp[:, 0:HW_]
                    )

        # Schraudolph fast-exp constants: round(S*SCA + SCB) as int16 IS the
        # bf16 bit pattern of ~exp(S) (max rel err 4.1%, std 1.8%, ~zero
        # mean).  Emitted on the Vector engine for DVE_STEPS of the 16 jt
        # steps so the Scalar engine (the per-step pacer) sheds 1/4 of the
        # exp stream.
        SCA = float(128.0 / np.log(2.0))
        SCB = float(127 * 128.0 - 7.3)
        SCA8 = float(8.0 / np.log(2.0))
        SCB8 = float(56.0 - 0.45)
        DVE_STEPS = (2, 6, 9, 13) if PV8 else ()

        def pair_ic(pair, ic, fillers, last=False):
            """Attention for head pair (2*pair, 2*pair+1) on query chunk ic.
            fillers: {jt: [callable, ...]} emitted just before that step.
            Software-pipelined: scores for jt+1 are emitted before the PV of
            jt, so the in-order PE queue never sits behind a PV that waits
            on a just-issued exp."""
            hA, hB = 2 * pair, 2 * pair + 1
            OA = op_.tile([128, IC], f32, tag="O")
            OB = op_.tile([128, IC], f32, tag="O")

            def s_step(jt, E, pl):
                Sp = ps.tile([128, 2 * IC], f32, tag="S")
                nc.tensor.matmul(
                    Sp[:, 0:IC],
                    kT[0:64, pair, jt * 128 : (jt + 1) * 128],
                    qT[0:64, pair, ic * IC : (ic + 1) * IC],
                    start=True,
                    stop=True,
                )
                nc.tensor.matmul(
                    Sp[:, IC : 2 * IC],
                    kT[64:128, pair, jt * 128 : (jt + 1) * 128],
                    qT[64:128, pair, ic * IC : (ic + 1) * IC],
                    start=True,
                    stop=True,
                )
                dst = E[:, pl, :] if PV8 else E[:]
                if jt in DVE_STEPS:
                    nc.vector.tensor_scalar(
                        dst.bitcast(i8 if PV8 else mybir.dt.int16), Sp[:],
                        SCA8 if PV8 else SCA, SCB8 if PV8 else SCB,
                        Alu.mult, Alu.add,
                    )
                else:
                    nc.scalar.activation(dst, Sp[:], Act.Exp)

            def new_E():
                if PV8:
                    E = ep.tile([128, 2, 2 * IC], f8, tag="E")
                else:
                    E = ep.tile([128, 2 * IC], bf16, tag="E")
                return E

            def pv_step(jt, E):
                # bf16: one j-tile per call.  fp8 DoubleRow: jt is a j-PAIR
                # index, E carries both j-planes, K=256 per matmul.
                if PV8:
                    lastp = ST // 2 - 1
                    nc.tensor.matmul(
                        OA[0:DH + 1, :],
                        vaug[:, jt, :, hA, 0 : DH + 1],
                        E[:, :, 0:IC],
                        start=(jt == 0),
                        stop=(jt == lastp),
                        perf_mode=DRMODE,
                    )
                    nc.tensor.matmul(
                        OB[0:DH + 1, :],
                        vaug[:, jt, :, hB, 0 : DH + 1],
                        E[:, :, IC : 2 * IC],
                        start=(jt == 0),
                        stop=(jt == lastp),
                        perf_mode=DRMODE,
                    )
                else:
                    nc.tensor.matmul(
                        OA[:],
                        vaug[:, jt, hA, :],
                        E[:, 0:IC],
                        start=(jt == 0),
                        stop=(jt == ST - 1),
                    )
                    nc.tensor.matmul(
                        OB[:],
                        vaug[:, jt, hB, :],
                        E[:, IC : 2 * IC],
                        start=(jt == 0),
                        stop=(jt == ST - 1),
                    )

            if PV8:
                # jt-pair pipeline: S(2jp)/S(2jp+1) fill the two planes of
                # E-tile jp; the DR PV for pair jp-1 is emitted two s_steps
                # later so exp always has slack.
                Etiles = {}
                for jt in range(ST):
                    for f in fillers.get(jt, ()):
                        f()
                    jp = jt // 2
                    if jt % 2 == 0:
                        Etiles[jp] = new_E()
                    s_step(jt, Etiles[jp], jt % 2)
                    if jt % 2 == 1 and jp >= 1:
                        pv_step(jp - 1, Etiles.pop(jp - 1))
                pv_step(ST // 2 - 1, Etiles.pop(ST // 2 - 1))
            else:
                for f in fillers.get(0, ()):
                    f()
                Eprev = new_E()
                s_step(0, Eprev, 0)
                for jt in range(1, ST):
                    for f in fillers.get(jt, ()):
                        f()
                    Ecur = new_E()
                    s_step(jt, Ecur, 0)
                    pv_step(jt - 1, Eprev)
                    Eprev = Ecur
                pv_step(ST - 1, Eprev)
            attn_norm(hA, ic, OA, pe_bcast=last)
            attn_norm(hB, ic, OB, pe_bcast=last)

        # ---- emission schedule ----
        qk_proj(1, 0, 0)
        qk_proj(0, 0, 0, alt=True)
        v_proj(0)
        v_proj(1)
        F = lambda *fs: list(fs)
        p0i0 = {jt: F(lambda st=jt + 2: v_proj(st)) for jt in range(ST - 2)}
        for jt, c in ((2, 1), (5, 2), (9, 3)):
            p0i0[jt] = [lambda c=c: qk_proj(1, 0, c)] + p0i0[jt]
        p0i0[12] = [lambda: qk_proj(0, 0, 1)] + p0i0[12]
        pair_ic(0, 0, p0i0)
        pair_ic(0, 1, {
            4: F(lambda: qk_proj(1, 1, 0)),
            6: F(lambda: qk_proj(1, 1, 1)),
            8: F(lambda: qk_proj(1, 1, 2)),
            10: F(lambda: qk_proj(1, 1, 3)),
            13: F(lambda: qk_proj(0, 1, 0)),
        })
        pair_ic(1, 0, {
            4: F(lambda: qk_proj(0, 1, 1)),
            9: F(lambda: qk_proj(0, 0, 2)),
        })
        pair_ic(1, 1, {
            4: F(lambda: qk_proj(0, 1, 2)),
            9: F(lambda: qk_proj(0, 0, 3)),
            12: F(lambda: qk_proj(0, 1, 3)),
        })
        pair_ic(0, 2, {5 + i: F(lambda st=(i + 2) // 2, oc=i % 2: o_proj_chunk(st, oc))
                       for i in range(6)})
        pair_ic(1, 2, {4 + i: F(lambda st=4 + i // 2, oc=i % 2: o_proj_chunk(st, oc))
                       for i in range(8)})
        pair_ic(0, 3, {4 + i: F(lambda st=8 + i // 2, oc=i % 2: o_proj_chunk(st, oc))
                       for i in range(8)})
        p13 = {4: F(lambda: o_proj_chunk(0, 0)),
               8: F(lambda: o_proj_chunk(0, 1))}
        slots = (5, 6, 7, 9, 10, 11, 12, 13)
        for i in range(8):
            st, oc = 12 + i // 2, i % 2
            p13.setdefault(slots[i], []).append(
                lambda st=st, oc=oc: o_proj_half(st, oc, 0))
        pair_ic(1, 3, p13, last=True)
        for st in range(12, ST):
            o_proj_half(st, 0, 1)
            o_proj_half(st, 1, 1)

    nc.compile()
    return nc


def _get_nc():
    global _nc_cache
    if _nc_cache is None:
        _nc_cache = _build_nc()
    return _nc_cache


def _prepare_in_maps(x, W_q, b_q, W_k, b_k, W_v, b_v, W_o, b_o):
    in_maps = []
    for c in range(N_CORES):
        b, g = c // 4, c % 4
        rows = slice(DL * g, DL * g + DL)
        bqk = np.stack(
            [
                b_q[DL * g : DL * g + 128],
                b_q[DL * g + 128 : DL * g + 256],
                b_k[DL * g : DL * g + 128],
                b_k[DL * g + 128 : DL * g + 256],
            ],
            axis=1,
        ).astype(np.float32)
        in_maps.append(
            {
                "xT": np.ascontiguousarray(x[b].T).astype(_BF16),
                "wq": np.ascontiguousarray(W_q[rows].T).astype(_BF16),
                "wk": np.ascontiguousarray(W_k[rows].T).astype(_BF16),
                "wv": np.ascontiguousarray(W_v[rows].T).astype(_BF16),
                "wo": np.ascontiguousarray(W_o[:, rows].T).astype(_BF16),
                "bqk": np.ascontiguousarray(bqk),
                "bv": np.ascontiguousarray(
                    np.broadcast_to(b_v[rows], (128, DL))
                ).astype(np.float32),
            }
        )
    return in_maps


def _assemble(results, b_o):
    out = np.empty((B, S, D), dtype=np.float32)
    for b in range(B):
        acc = results[4 * b]["out"].astype(np.float32).copy()
        acc[12 * 128 :] += results[4 * b]["out2"]
        for g in range(1, 4):
            acc += results[4 * b + g]["out"]
            acc[12 * 128 :] += results[4 * b + g]["out2"]
        out[b] = acc + b_o[None, :].astype(np.float32)
    return out


def kernel(x, W_q, b_q, W_k, b_k, W_v, b_v, W_o, b_o):
    from concourse.bass_utils import run_bass_kernel_spmd

    x = np.asarray(x, dtype=np.float32)
    nc = _get_nc()
    in_maps = _prepare_in_maps(
        x,
        np.asarray(W_q, np.float32),
        np.asarray(b_q, np.float32),
        np.asarray(W_k, np.float32),
        np.asarray(b_k, np.float32),
        np.asarray(W_v, np.float32),
        np.asarray(b_v, np.float32),
        np.asarray(W_o, np.float32),
        np.asarray(b_o, np.float32),
    )
    res = run_bass_kernel_spmd(nc, in_maps, core_ids=list(range(N_CORES)))
    return _assemble(res.results, np.asarray(b_o, np.float32))

